# revision 39
# baseline (speedup 1.0000x reference)
"""Trainium2 Bass kernel for nn_Attention_61040075210876.

Full multi-head attention block: qkv = x @ W_qkv, RoPE on q/k,
attn = softmax(q k^T / sqrt(D)), ctx = attn @ v, out = ctx @ W_out + b_out.
Returns (out, attn) like the reference.

Sharding over 8 NeuronCores: core c handles batch b = c//4 and the 4 heads
h in [4*(c%4), 4*(c%4)+4). Per core:
  - qkv projection only for its heads (W_qkv column shard; q-columns
    pre-scaled by D**-0.5; q/k columns permuted per head to [evens|odds]
    so RoPE pair-partners sit 32 partitions apart in feature-major layout)
  - scores computed TWICE on the tensor engine (query-major for the attn
    output, with row-sum Z from the exp activation's accum_out; key-major
    for the attn @ v contraction, with 64 ones-columns appended to v so the
    same matmul replicates Z across partitions 64:128)
  - the attn@v accumulation runs as two closed 8-matmul PSUM groups (HW
    mis-accumulates when an accumulation group interleaves with other
    matmul groups, though CoreSim accepts it); the per-key reciprocal runs
    at base partition 0 after a DMA lane shift (custom-DVE ops at base 64
    also misbehaved on HW)
  - out partial = ctx @ W_out(row shard); host sums partials + bias.

All heavy matmuls run in float32r (fp32 data rounded to fp32r by the
producing instruction; ~1.5e-4 rel err, 4x faster than true fp32).
"""
import sys
import os
from contextlib import ExitStack
from dataclasses import dataclass

sys.path.insert(0, "/opt/trn_rl_repo")

import numpy as np

import concourse.bass as bass  # noqa: E402
import concourse.tile as tile  # noqa: E402
from concourse import bacc, mybir  # noqa: E402
from concourse.masks import make_identity  # noqa: E402

F32 = mybir.dt.float32
F32R = mybir.dt.float32r
BF16 = mybir.dt.bfloat16
AF = mybir.ActivationFunctionType


@dataclass(frozen=True)
class Cfg:
    N: int = 2048      # sequence length
    D: int = 1024      # model dim
    NH: int = 4        # heads per core
    HD: int = 64       # head dim
    n_cores: int = 8

    @property
    def DH(self):      # features per core for q/k/v
        return self.NH * self.HD

    @property
    def KT(self):      # 128-row tiles of D
        return self.D // 128

    @property
    def MT(self):      # 128-row tiles of N
        return self.N // 128

    @property
    def NG(self):      # 128-row feature groups (2 heads each)
        return self.DH // 128


CFG = Cfg()


def build_bass(cfg: Cfg, debug: bool = False, dbg_dump: bool = False, reps: int = 1):
    nc = bacc.Bacc("TRN2", target_bir_lowering=False, debug=debug)
    N, D, NH, HD = cfg.N, cfg.D, cfg.NH, cfg.HD
    DH, KT, MT, NG = cfg.DH, cfg.KT, cfg.MT, cfg.NG
    assert N % 256 == 0 and D % 128 == 0 and HD == 64 and NH % 2 == 0

    xb_d = nc.dram_tensor("xb", [N, D], F32, kind="ExternalInput").ap()
    wq_d = nc.dram_tensor("wq", [D, DH], F32, kind="ExternalInput").ap()
    wk_d = nc.dram_tensor("wk", [D, DH], F32, kind="ExternalInput").ap()
    wv_d = nc.dram_tensor("wv", [D, DH], F32, kind="ExternalInput").ap()
    wo_d = nc.dram_tensor("wo", [DH, D], F32, kind="ExternalInput").ap()
    rcos_d = nc.dram_tensor("rcos", [128, N], F32, kind="ExternalInput").ap()
    rsin_d = nc.dram_tensor("rsin", [128, N], F32, kind="ExternalInput").ap()
    ones_d = nc.dram_tensor("ones", [128, 256], F32, kind="ExternalInput").ap()
    attn_d = nc.dram_tensor("attn4", [NH, N, N], F32, kind="ExternalOutput").ap()
    outp_d = nc.dram_tensor("outp", [N, D], F32, kind="ExternalOutput").ap()
    if dbg_dump:
        qT_d = nc.dram_tensor("qT_dbg", [128, cfg.NG, N], F32, kind="ExternalOutput").ap()
        kT_d = nc.dram_tensor("kT_dbg", [128, cfg.NG, N], F32, kind="ExternalOutput").ap()
        v_d = nc.dram_tensor("v_dbg", [128, cfg.MT, NH, 2 * HD], F32, kind="ExternalOutput").ap()
        ctx_d = nc.dram_tensor("ctx_dbg", [128, cfg.NG, N], F32, kind="ExternalOutput").ap()

    with tile.TileContext(nc) as tc, ExitStack() as octx:
      for _rep in range(reps):
        ctx = octx.enter_context(ExitStack())
        # ---- persistent tiles ----
        pers = ctx.enter_context(tc.tile_pool(name="pers", bufs=1))
        qT = pers.tile([128, NG, N], F32R, name="qT")     # rope'd, feature-major
        kT = pers.tile([128, NG, N], F32R, name="kT")
        v_sb = pers.tile([128, MT, NH, 2 * HD], F32R, name="v_sb")
        ctxT = pers.tile([128, NG, N], F32R, name="ctxT")
        wo_sb = pers.tile([128, NG, D], F32R, name="wo_sb")

        for _mt in range(MT):
            nc.gpsimd.dma_start(
                v_sb[:, _mt, :, HD:2 * HD],
                ones_d.rearrange("p (h d) -> p h d", d=64))

        # ---- stage 1+2+3: x transpose (halves), v, per-group q/k + rope ----
        with ExitStack() as sctx:
            s_in = sctx.enter_context(tc.tile_pool(name="s_in", bufs=3))
            s_w = sctx.enter_context(tc.tile_pool(name="s_w", bufs=1))
            s_xt = sctx.enter_context(tc.tile_pool(name="s_xt", bufs=1))
            s_tmp = sctx.enter_context(tc.tile_pool(name="s_tmp", bufs=2))
            p_tr = sctx.enter_context(tc.tile_pool(name="p_tr", bufs=4, space="PSUM"))
            p_qk = sctx.enter_context(tc.tile_pool(name="p_qk", bufs=2, space="PSUM"))
            p_v = sctx.enter_context(tc.tile_pool(name="p_v", bufs=2, space="PSUM"))

            ident = s_w.tile([128, 128], F32, name="ident")
            make_identity(nc, ident)

            wq_sb = s_w.tile([128, KT, DH], F32R, name="wq_sb")
            wk_sb = s_w.tile([128, KT, DH], F32R, name="wk_sb")
            wv_sb = s_w.tile([128, KT, DH], F32R, name="wv_sb")
            rcos = s_w.tile([128, N], F32R, name="rcos")
            rsin = s_w.tile([128, N], F32R, name="rsin")
            for w_d, w_sb in ((wv_d, wv_sb), (wq_d, wq_sb), (wk_d, wk_sb)):
                wstg = s_w.tile([128, KT, DH], F32, name="wstg", tag="wstg")
                nc.sync.dma_start(wstg[:], w_d.rearrange("(t p) n -> p t n", p=128))
                nc.vector.tensor_copy(w_sb[:], wstg[:])
            nc.gpsimd.dma_start(rcos[:], rcos_d[:])
            nc.gpsimd.dma_start(rsin[:], rsin_d[:])

            xb_r = xb_d.rearrange("(mt p) d -> mt p d", p=128)
            NH2 = N // 2
            MH = MT // 2

            def rope_group(dstT, g):
                # in place: t = t*cos + swap32(t*sin')  (sin' pre-swapped signs)
                for h0 in range(0, N, N // 2):
                    cols = slice(h0, h0 + N // 2)
                    dst = dstT[:, g, cols]
                    tmp = s_tmp.tile([128, N // 2], F32R, name="tmp", tag="tmp")
                    tmp2 = s_tmp.tile([128, N // 2], F32R, name="tmp2",
                                      tag="tmp2")
                    nc.vector.tensor_mul(tmp[:], dst, rsin[:, cols])
                    for s in range(4):
                        lo, hi = s * 32, (s + 1) * 32
                        plo, phi = (s ^ 1) * 32, ((s ^ 1) + 1) * 32
                        nc.sync.dma_start(tmp2[lo:hi, :], tmp[plo:phi, :])
                    nc.vector.tensor_mul(dst, dst, rcos[:, cols])
                    nc.vector.tensor_add(dst, dst, tmp2[:])

            for half in range(2):
                xTh = s_xt.tile([128, KT, NH2], F32R, name="xTh", tag="xTh")
                for mt in range(MH):
                    gmt = half * MH + mt
                    xrow = s_in.tile([128, D], F32, name="xrow", tag="xrow")
                    nc.sync.dma_start(xrow[:], xb_r[gmt])
                    for dt0 in range(0, KT, 4):
                        nd = min(4, KT - dt0)
                        ptr = p_tr.tile([128, 512], F32, name="ptr")
                        for dj in range(nd):
                            nc.tensor.transpose(
                                ptr[:, dj * 128:(dj + 1) * 128],
                                xrow[:, (dt0 + dj) * 128:(dt0 + dj + 1) * 128],
                                ident[:])
                        nc.vector.tensor_copy(
                            xTh[:, dt0:dt0 + nd, mt * 128:(mt + 1) * 128],
                            ptr[:, 0:nd * 128].rearrange("p (d c) -> p d c",
                                                         c=128))

                IC = min(512, NH2)

                def qk_group(g):
                    for w_sb, dstT in ((wk_sb, kT), (wq_sb, qT)):
                        for ic in range(NH2 // IC):
                            pq = p_qk.tile([128, 512], F32, name="pq")
                            for t in range(KT):
                                nc.tensor.matmul(
                                    pq[:, 0:IC],
                                    w_sb[:, t, g * 128:(g + 1) * 128],
                                    xTh[:, t, ic * IC:(ic + 1) * IC],
                                    start=(t == 0), stop=(t == KT - 1))
                            nc.vector.tensor_copy(
                                dstT[:, g, half * NH2 + ic * IC:
                                     half * NH2 + (ic + 1) * IC], pq[:, 0:IC])
                    if half == 1:
                        rope_group(kT, g)
                        rope_group(qT, g)

                # g0 first so its rope (gating stage 4) lands earliest;
                # v before g1 (AV needs it early in phase A)
                qk_group(0)
                for mt in range(MH):
                    gmt = half * MH + mt
                    pv = p_v.tile([128, DH], F32, name="pv")
                    for t in range(KT):
                        nc.tensor.matmul(
                            pv[:], xTh[:, t, mt * 128:(mt + 1) * 128],
                            wv_sb[:, t, :], start=(t == 0), stop=(t == KT - 1))
                    nc.vector.tensor_copy(
                        v_sb[:, gmt, :, 0:HD],
                        pv[:].rearrange("p (h d) -> p h d", d=HD))
                for g in range(1, NG):
                    qk_group(g)

        # ---- stage 4: attention, pairs of a group interleaved for PE
        # row/col-group packing (even pair on partitions 0:64, odd on 64:128)
        s_et = ctx.enter_context(tc.tile_pool(name="s_et", bufs=1))
        s_p = ctx.enter_context(tc.tile_pool(name="s_p", bufs=2))
        s_z = ctx.enter_context(tc.tile_pool(name="s_z", bufs=2))
        p_s = ctx.enter_context(tc.tile_pool(name="p_s", bufs=2, space="PSUM"))
        p_av = ctx.enter_context(tc.tile_pool(name="p_av", bufs=1, space="PSUM"))

        NJT = MT                  # key-major j-tiles (128 wide)
        NIB = N // 512            # key-major i-blocks
        NJG = (N + 1023) // 1024  # 1024-wide psum slabs per row


        def prs_of(g):
            return [(2 * g, 0), (2 * g + 1, 64)]

        def phase_a_ib(g, ib):
            prs = prs_of(g)
            i0 = ib * 512
            NSEG = 2
            seg_len = NJT // NSEG
            # expT for one segment of j-tiles, per pair
            ets = {p: [] for p, _ in prs}
            avsegs = {p: [] for p, _ in prs}
            for seg in range(NSEG):
                for p, r0 in prs:
                    ets[p].append(s_et.tile([128, seg_len * 512], F32R,
                                            name="et", tag=f"et{r0}"))
                for jt0 in range(seg * seg_len, (seg + 1) * seg_len, 2):
                    nj = min(2, (seg + 1) * seg_len - jt0)
                    slabs = {}
                    for p, r0 in prs:
                        slabs[p] = p_s.tile([128, 1024], F32, name="km",
                                            tag="slab")
                    for j in range(nj):
                        jt = jt0 + j
                        for p, r0 in prs:
                            nc.tensor.matmul(
                                slabs[p][:, j * 512:(j + 1) * 512],
                                kT[r0:r0 + 64, g, jt * 128:(jt + 1) * 128],
                                qT[r0:r0 + 64, g, i0:i0 + 512],
                                start=True, stop=True)
                    for p, r0 in prs:
                        c0 = (jt0 - seg * seg_len) * 512
                        nc.scalar.activation(ets[p][-1][:, c0:c0 + nj * 512],
                                             slabs[p][:, 0:nj * 512], AF.Exp)
                # closed accumulation group: seg_len consecutive matmuls
                for p, r0 in prs:
                    av = p_av.tile([128, 512], F32, name="av",
                                   tag=f"av{r0}_{seg}")
                    avsegs[p].append(av)
                    for jj in range(seg_len):
                        jt = seg * seg_len + jj
                        nc.tensor.matmul(av[:], v_sb[:, jt, p, :],
                                         ets[p][-1][:, jj * 512:(jj + 1) * 512],
                                         start=(jj == 0),
                                         stop=(jj == seg_len - 1))
            for p, r0 in prs:
                # combine segments in SBUF, reciprocal at base 0 (DMA shift)
                zc = s_z.tile([128, 512], F32, name="zc", tag=f"zc{r0}")
                nc.vector.tensor_copy(zc[:], avsegs[p][0][:])
                nc.vector.tensor_add(zc[:], zc[:], avsegs[p][1][:])
                zlo = s_z.tile([64, 512], F32, name="zlo", tag=f"zlo{r0}")
                nc.sync.dma_start(zlo[:], zc[64:128, :])
                rzlo = s_z.tile([64, 512], F32, name="rzlo", tag=f"rzlo{r0}")
                nc.vector.reciprocal_approx_fast(rzlo[:], zlo[:])
                if r0 == 0:
                    nc.vector.tensor_mul(ctxT[0:64, g, i0:i0 + 512],
                                         zc[0:64, :], rzlo[:])
                else:
                    cstg = s_z.tile([64, 512], F32R, name="cstg", tag="cstg")
                    nc.vector.tensor_mul(cstg[:], zc[0:64, :], rzlo[:])
                    nc.sync.dma_start(ctxT[64:128, g, i0:i0 + 512], cstg[:])

        def phase_b_it(g, it):
            prs = prs_of(g)
            i0 = it * 128
            Ps, zqps = {}, {}
            for p, r0 in prs:
                Ps[p] = s_p.tile([128, N], F32, name="P", tag=f"P{r0}")
                zqps[p] = s_z.tile([128, NJG], F32, name="zqp", tag=f"zqp{r0}")
            for jg in range(NJG):
                j0 = jg * 1024
                w = min(1024, N - j0)
                qms = {}
                for p, r0 in prs:
                    qms[p] = p_s.tile([128, 1024], F32, name="qm", tag="slab")
                for j in range(w // 512):
                    for p, r0 in prs:
                        nc.tensor.matmul(
                            qms[p][:, j * 512:(j + 1) * 512],
                            qT[r0:r0 + 64, g, i0:i0 + 128],
                            kT[r0:r0 + 64, g, j0 + j * 512:j0 + (j + 1) * 512],
                            start=True, stop=True)
                for p, r0 in prs:
                    nc.scalar.activation(Ps[p][:, j0:j0 + w],
                                         qms[p][:, 0:w], AF.Exp,
                                         accum_out=zqps[p][:, jg:jg + 1])
            for p, r0 in prs:
                zq = s_z.tile([128, 1], F32, name="zq", tag=f"zq{r0}")
                rzq = s_z.tile([128, 1], F32, name="rzq", tag=f"rzq{r0}")
                if NJG > 1:
                    nc.vector.tensor_reduce(zq[:], zqps[p][:],
                                            mybir.AxisListType.X,
                                            mybir.AluOpType.add)
                else:
                    zq = zqps[p]
                nc.vector.reciprocal(rzq[:], zq[:])
                nc.vector.tensor_scalar_mul(Ps[p][:], Ps[p][:], rzq[:])
                nc.sync.dma_start(attn_d[p, i0:i0 + 128, :], Ps[p][:])

        # schedule: per group, interleave A-ibs with B-its (B is independent
        # of A thanks to accum_out Z); wout rides with group 1
        for step in range(NIB):
            phase_a_ib(0, step)
            for it in range(step * (MT // NIB), (step + 1) * (MT // NIB)):
                phase_b_it(0, it)

        # ---- stage 5 interleaved into B(1): out partial = ctxT.T @ wo ----
        nc.gpsimd.dma_start(wo_sb[:], wo_d.rearrange("(g p) n -> p g n", p=128))
        s_o = ctx.enter_context(tc.tile_pool(name="s_o", bufs=3))

        def wout_it(it):
            i0 = it * 128
            for nt in range((D + 511) // 512):
                nf = min(512, D - nt * 512)
                po = p_av.tile([128, 512], F32, name="po", tag="av0_0")
                for ct in range(NG):
                    nc.tensor.matmul(po[:, 0:nf], ctxT[:, ct, i0:i0 + 128],
                                     wo_sb[:, ct, nt * 512:nt * 512 + nf],
                                     start=(ct == 0), stop=(ct == NG - 1))
                osb = s_o.tile([128, 512], F32, name="osb", tag="osb")
                nc.vector.tensor_copy(osb[:, 0:nf], po[:, 0:nf])
                nc.sync.dma_start(outp_d[i0:i0 + 128, nt * 512:nt * 512 + nf],
                                  osb[:, 0:nf])

        for step in range(NIB):
            phase_a_ib(1, step)
            for it in range(step * (MT // NIB), (step + 1) * (MT // NIB)):
                phase_b_it(1, it)
                wout_it(it)

        if dbg_dump:
            nc.gpsimd.dma_start(qT_d[:], qT[:])
            nc.gpsimd.dma_start(kT_d[:], kT[:])
            nc.gpsimd.dma_start(v_d[:], v_sb[:])
            nc.gpsimd.dma_start(ctx_d[:], ctxT[:])

        ctx.close()

    nc.compile()
    return nc


def host_prep(cfg: Cfg, x, freqs_cis, W_qkv, W_out):
    """Build per-core input maps."""
    N, D, NH, HD = cfg.N, cfg.D, cfg.NH, cfg.HD
    x = np.ascontiguousarray(np.asarray(x, dtype=np.float32))
    freqs_cis = np.asarray(freqs_cis, dtype=np.float32)
    W_qkv = np.asarray(W_qkv, dtype=np.float32)
    W_out = np.asarray(W_out, dtype=np.float32)
    scale = float(D) ** -0.5
    D2 = HD // 2

    # rope multiplier tiles [128, N]: 4 blocks of 32 rows
    cosT = np.ascontiguousarray(freqs_cis[:, :, 0].T)   # [D2=32, N]
    sinT = np.ascontiguousarray(freqs_cis[:, :, 1].T)
    assert D2 == 32
    rcos = np.ascontiguousarray(np.tile(cosT, (4, 1)))
    # pre-swapped sign layout: after multiplying, a +-32-partition block swap
    # (done via SBUF->SBUF DMA) lands each product on its target rows
    rsin = np.ascontiguousarray(np.concatenate([sinT, -sinT, sinT, -sinT], axis=0))

    perm = np.concatenate([np.arange(0, HD, 2), np.arange(1, HD, 2)])  # evens|odds

    total_heads = W_qkv.shape[1] // (3 * HD) if False else (
        W_qkv.shape[1] // 3 // HD)
    n_hg = total_heads // NH  # head groups per batch
    in_maps = []
    for c in range(cfg.n_cores):
        b, hg = divmod(c, n_hg)
        heads = range(hg * NH, (hg + 1) * NH)
        qcols, kcols, vcols, orows = [], [], [], []
        for h in heads:
            base = h * HD
            qcols.append(base + perm)
            kcols.append(D + base + perm)
            vcols.append(2 * D + base + np.arange(HD))
            orows.append(base + np.arange(HD))
        qcols = np.concatenate(qcols)
        kcols = np.concatenate(kcols)
        vcols = np.concatenate(vcols)
        orows = np.concatenate(orows)
        in_maps.append({
            "xb": x[b],
            "wq": np.ascontiguousarray(W_qkv[:, qcols] * scale),
            "wk": np.ascontiguousarray(W_qkv[:, kcols]),
            "wv": np.ascontiguousarray(W_qkv[:, vcols]),
            "wo": np.ascontiguousarray(W_out[orows, :]),
            "rcos": rcos,
            "rsin": rsin,
            "ones": np.ones((128, 256), dtype=np.float32),
        })
    return in_maps


_NC_CACHE = {}


def kernel(x, freqs_cis, W_qkv, W_out, b_out):
    from concourse.bass_utils import run_bass_kernel_spmd

    cfg = CFG
    in_maps = host_prep(cfg, x, freqs_cis, W_qkv, W_out)

    if "nc" not in _NC_CACHE:
        _NC_CACHE["nc"] = build_bass(cfg)
    nc = _NC_CACHE["nc"]

    kwargs = {}
    if os.environ.get("BASS_KERNEL_TRACE"):
        kwargs = dict(trace=True, trace_cores=[0])
    res = run_bass_kernel_spmd(nc, in_maps, core_ids=list(range(cfg.n_cores)),
                               **kwargs)
    _NC_CACHE["last_results"] = res

    N, D, NH = cfg.N, cfg.D, cfg.NH
    n_hg = 16 // NH
    b_out = np.asarray(b_out, dtype=np.float32)
    attn = np.empty((2, 16, N, N), dtype=np.float32)
    out = np.zeros((2, N, D), dtype=np.float32)
    for c in range(cfg.n_cores):
        b, hg = divmod(c, n_hg)
        r = res.results[c]
        attn[b, hg * NH:(hg + 1) * NH] = r["attn4"]
        out[b] += r["outp"]
    out += b_out
    return (out, attn)


# revision 44
# speedup vs baseline: 1.0069x; 1.0069x over previous
"""Trainium2 Bass kernel for nn_Attention_61040075210876.

Full multi-head attention block: qkv = x @ W_qkv, RoPE on q/k,
attn = softmax(q k^T / sqrt(D)), ctx = attn @ v, out = ctx @ W_out + b_out.
Returns (out, attn) like the reference.

Sharding over 8 NeuronCores: core c handles batch b = c//4 and the 4 heads
h in [4*(c%4), 4*(c%4)+4). Per core:
  - qkv projection only for its heads (W_qkv column shard; q-columns
    pre-scaled by D**-0.5; q/k columns permuted per head to [evens|odds]
    so RoPE pair-partners sit 32 partitions apart in feature-major layout)
  - scores computed TWICE on the tensor engine (query-major for the attn
    output, with row-sum Z from the exp activation's accum_out; key-major
    for the attn @ v contraction, with 64 ones-columns appended to v so the
    same matmul replicates Z across partitions 64:128)
  - the attn@v accumulation runs as two closed 8-matmul PSUM groups (HW
    mis-accumulates when an accumulation group interleaves with other
    matmul groups, though CoreSim accepts it); the per-key reciprocal runs
    at base partition 0 after a DMA lane shift (custom-DVE ops at base 64
    also misbehaved on HW)
  - out partial = ctx @ W_out(row shard); host sums partials + bias.

All heavy matmuls run in float32r (fp32 data rounded to fp32r by the
producing instruction; ~1.5e-4 rel err, 4x faster than true fp32).
"""
import sys
import os
from contextlib import ExitStack
from dataclasses import dataclass

sys.path.insert(0, "/opt/trn_rl_repo")

import numpy as np

import concourse.bass as bass  # noqa: E402
import concourse.tile as tile  # noqa: E402
from concourse import bacc, mybir  # noqa: E402
from concourse.masks import make_identity  # noqa: E402

F32 = mybir.dt.float32
F32R = mybir.dt.float32r
BF16 = mybir.dt.bfloat16
AF = mybir.ActivationFunctionType


@dataclass(frozen=True)
class Cfg:
    N: int = 2048      # sequence length
    D: int = 1024      # model dim
    NH: int = 4        # heads per core
    HD: int = 64       # head dim
    n_cores: int = 8

    @property
    def DH(self):      # features per core for q/k/v
        return self.NH * self.HD

    @property
    def KT(self):      # 128-row tiles of D
        return self.D // 128

    @property
    def MT(self):      # 128-row tiles of N
        return self.N // 128

    @property
    def NG(self):      # 128-row feature groups (2 heads each)
        return self.DH // 128


CFG = Cfg()


def build_bass(cfg: Cfg, debug: bool = False, dbg_dump: bool = False, reps: int = 1):
    nc = bacc.Bacc("TRN2", target_bir_lowering=False, debug=debug)
    N, D, NH, HD = cfg.N, cfg.D, cfg.NH, cfg.HD
    DH, KT, MT, NG = cfg.DH, cfg.KT, cfg.MT, cfg.NG
    assert N % 256 == 0 and D % 128 == 0 and HD == 64 and NH % 2 == 0

    xb_d = nc.dram_tensor("xb", [N, D], F32, kind="ExternalInput").ap()
    wq_d = nc.dram_tensor("wq", [D, DH], F32, kind="ExternalInput").ap()
    wk_d = nc.dram_tensor("wk", [D, DH], F32, kind="ExternalInput").ap()
    wv_d = nc.dram_tensor("wv", [D, DH], F32, kind="ExternalInput").ap()
    wo_d = nc.dram_tensor("wo", [DH, D], F32, kind="ExternalInput").ap()
    rcos_d = nc.dram_tensor("rcos", [128, N], F32, kind="ExternalInput").ap()
    rsin_d = nc.dram_tensor("rsin", [128, N], F32, kind="ExternalInput").ap()
    ones_d = nc.dram_tensor("ones", [128, 256], F32, kind="ExternalInput").ap()
    attn_d = nc.dram_tensor("attn4", [NH, N, N], F32, kind="ExternalOutput").ap()
    outp_d = nc.dram_tensor("outp", [N, D], F32, kind="ExternalOutput").ap()
    if dbg_dump:
        qT_d = nc.dram_tensor("qT_dbg", [128, cfg.NG, N], F32, kind="ExternalOutput").ap()
        kT_d = nc.dram_tensor("kT_dbg", [128, cfg.NG, N], F32, kind="ExternalOutput").ap()
        v_d = nc.dram_tensor("v_dbg", [128, cfg.MT, NH, 2 * HD], F32, kind="ExternalOutput").ap()
        ctx_d = nc.dram_tensor("ctx_dbg", [128, cfg.NG, N], F32, kind="ExternalOutput").ap()

    with tile.TileContext(nc) as tc, ExitStack() as octx:
      for _rep in range(reps):
        ctx = octx.enter_context(ExitStack())
        # ---- persistent tiles ----
        pers = ctx.enter_context(tc.tile_pool(name="pers", bufs=1))
        qT = pers.tile([128, NG, N], F32R, name="qT")     # rope'd, feature-major
        kT = pers.tile([128, NG, N], F32R, name="kT")
        v_sb = pers.tile([128, MT, NH, 2 * HD], F32R, name="v_sb")
        ctxT = pers.tile([128, NG, N], F32R, name="ctxT")
        wo_sb = pers.tile([128, NG, D], F32R, name="wo_sb")

        for _mt in range(MT):
            nc.gpsimd.dma_start(
                v_sb[:, _mt, :, HD:2 * HD],
                ones_d.rearrange("p (h d) -> p h d", d=64))

        # ---- stage 1+2+3: x transpose (halves), v, per-group q/k + rope ----
        with ExitStack() as sctx:
            s_in = sctx.enter_context(tc.tile_pool(name="s_in", bufs=3))
            s_w = sctx.enter_context(tc.tile_pool(name="s_w", bufs=1))
            s_xt = sctx.enter_context(tc.tile_pool(name="s_xt", bufs=1))
            s_tmp = sctx.enter_context(tc.tile_pool(name="s_tmp", bufs=2))
            p_tr = sctx.enter_context(tc.tile_pool(name="p_tr", bufs=4, space="PSUM"))
            p_qk = sctx.enter_context(tc.tile_pool(name="p_qk", bufs=2, space="PSUM"))
            p_v = sctx.enter_context(tc.tile_pool(name="p_v", bufs=2, space="PSUM"))

            ident = s_w.tile([128, 128], F32, name="ident")
            make_identity(nc, ident)

            wq_sb = s_w.tile([128, KT, DH], F32R, name="wq_sb")
            wk_sb = s_w.tile([128, KT, DH], F32R, name="wk_sb")
            wv_sb = s_w.tile([128, KT, DH], F32R, name="wv_sb")
            rcos = s_w.tile([128, N], F32R, name="rcos")
            rsin = s_w.tile([128, N], F32R, name="rsin")
            for w_d, w_sb in ((wv_d, wv_sb), (wq_d, wq_sb), (wk_d, wk_sb)):
                wstg = s_w.tile([128, KT, DH], F32, name="wstg", tag="wstg")
                nc.sync.dma_start(wstg[:], w_d.rearrange("(t p) n -> p t n", p=128))
                nc.vector.tensor_copy(w_sb[:], wstg[:])
            nc.gpsimd.dma_start(rcos[:], rcos_d[:])
            nc.gpsimd.dma_start(rsin[:], rsin_d[:])

            xb_r = xb_d.rearrange("(mt p) d -> mt p d", p=128)
            NH2 = N // 2
            MH = MT // 2

            def rope_group(dstT, g):
                # in place: t = t*cos + swap32(t*sin')  (sin' pre-swapped signs)
                for h0 in range(0, N, N // 2):
                    cols = slice(h0, h0 + N // 2)
                    dst = dstT[:, g, cols]
                    tmp = s_tmp.tile([128, N // 2], F32R, name="tmp", tag="tmp")
                    tmp2 = s_tmp.tile([128, N // 2], F32R, name="tmp2",
                                      tag="tmp2")
                    nc.vector.tensor_mul(tmp[:], dst, rsin[:, cols])
                    for s in range(4):
                        lo, hi = s * 32, (s + 1) * 32
                        plo, phi = (s ^ 1) * 32, ((s ^ 1) + 1) * 32
                        nc.sync.dma_start(tmp2[lo:hi, :], tmp[plo:phi, :])
                    nc.vector.tensor_mul(dst, dst, rcos[:, cols])
                    nc.vector.tensor_add(dst, dst, tmp2[:])

            for half in range(2):
                xTh = s_xt.tile([128, KT, NH2], F32R, name="xTh", tag="xTh")
                for mt in range(MH):
                    gmt = half * MH + mt
                    xrow = s_in.tile([128, D], F32, name="xrow", tag="xrow")
                    nc.sync.dma_start(xrow[:], xb_r[gmt])
                    for dt0 in range(0, KT, 4):
                        nd = min(4, KT - dt0)
                        ptr = p_tr.tile([128, 512], F32, name="ptr")
                        for dj in range(nd):
                            nc.tensor.transpose(
                                ptr[:, dj * 128:(dj + 1) * 128],
                                xrow[:, (dt0 + dj) * 128:(dt0 + dj + 1) * 128],
                                ident[:])
                        nc.vector.tensor_copy(
                            xTh[:, dt0:dt0 + nd, mt * 128:(mt + 1) * 128],
                            ptr[:, 0:nd * 128].rearrange("p (d c) -> p d c",
                                                         c=128))

                IC = min(512, NH2)

                def qk_group(g):
                    for w_sb, dstT in ((wk_sb, kT), (wq_sb, qT)):
                        for ic in range(NH2 // IC):
                            pq = p_qk.tile([128, 512], F32, name="pq")
                            for t in range(KT):
                                nc.tensor.matmul(
                                    pq[:, 0:IC],
                                    w_sb[:, t, g * 128:(g + 1) * 128],
                                    xTh[:, t, ic * IC:(ic + 1) * IC],
                                    start=(t == 0), stop=(t == KT - 1))
                            nc.vector.tensor_copy(
                                dstT[:, g, half * NH2 + ic * IC:
                                     half * NH2 + (ic + 1) * IC], pq[:, 0:IC])
                    if half == 1:
                        rope_group(kT, g)
                        rope_group(qT, g)

                # g0 first so its rope (gating stage 4) lands earliest;
                # v before g1 (AV needs it early in phase A)
                qk_group(0)
                for mt in range(MH):
                    gmt = half * MH + mt
                    pv = p_v.tile([128, DH], F32, name="pv")
                    for t in range(KT):
                        nc.tensor.matmul(
                            pv[:], xTh[:, t, mt * 128:(mt + 1) * 128],
                            wv_sb[:, t, :], start=(t == 0), stop=(t == KT - 1))
                    nc.vector.tensor_copy(
                        v_sb[:, gmt, :, 0:HD],
                        pv[:].rearrange("p (h d) -> p h d", d=HD))
                for g in range(1, NG):
                    qk_group(g)

        # ---- stage 4: attention, pairs of a group interleaved for PE
        # row/col-group packing (even pair on partitions 0:64, odd on 64:128)
        s_et = ctx.enter_context(tc.tile_pool(name="s_et", bufs=1))
        s_p = ctx.enter_context(tc.tile_pool(name="s_p", bufs=2))
        s_z = ctx.enter_context(tc.tile_pool(name="s_z", bufs=2))
        p_s = ctx.enter_context(tc.tile_pool(name="p_s", bufs=3, space="PSUM"))
        p_av = ctx.enter_context(tc.tile_pool(name="p_av", bufs=1, space="PSUM"))

        NJT = MT                  # key-major j-tiles (128 wide)
        NIB = N // 512            # key-major i-blocks
        NJG = (N + 1023) // 1024  # 1024-wide psum slabs per row


        def prs_of(g):
            return [(2 * g, 0), (2 * g + 1, 64)]

        def phase_a_ib(g, ib):
            prs = prs_of(g)
            i0 = ib * 512
            NSEG = 2
            seg_len = NJT // NSEG
            # expT for one segment of j-tiles, per pair
            ets = {p: [] for p, _ in prs}
            avsegs = {p: [] for p, _ in prs}
            zcs = {}
            for p, r0 in prs:
                zcs[p] = s_z.tile([128, 512], F32, name="zc", tag=f"zc{r0}")
            for seg in range(NSEG):
                for p, r0 in prs:
                    ets[p].append(s_et.tile([128, seg_len * 512], F32R,
                                            name="et", tag=f"et{r0}"))
                for jt0 in range(seg * seg_len, (seg + 1) * seg_len, 2):
                    nj = min(2, (seg + 1) * seg_len - jt0)
                    slabs = {}
                    for p, r0 in prs:
                        slabs[p] = p_s.tile([128, 1024], F32, name="km",
                                            tag="slab")
                    for j in range(nj):
                        jt = jt0 + j
                        for p, r0 in prs:
                            nc.tensor.matmul(
                                slabs[p][:, j * 512:(j + 1) * 512],
                                kT[r0:r0 + 64, g, jt * 128:(jt + 1) * 128],
                                qT[r0:r0 + 64, g, i0:i0 + 512],
                                start=True, stop=True)
                    for p, r0 in prs:
                        c0 = (jt0 - seg * seg_len) * 512
                        nc.scalar.activation(ets[p][-1][:, c0:c0 + nj * 512],
                                             slabs[p][:, 0:nj * 512], AF.Exp)
                # closed accumulation group: seg_len consecutive matmuls,
                # folded into the SBUF accumulator right away so the av psum
                # slot recycles between segments (frees 2 banks for slabs)
                for p, r0 in prs:
                    av = p_av.tile([128, 512], F32, name="av", tag=f"av{r0}")
                    avsegs[p].append(av)
                    for jj in range(seg_len):
                        jt = seg * seg_len + jj
                        nc.tensor.matmul(av[:], v_sb[:, jt, p, :],
                                         ets[p][-1][:, jj * 512:(jj + 1) * 512],
                                         start=(jj == 0),
                                         stop=(jj == seg_len - 1))
                    if seg == 0:
                        nc.vector.tensor_copy(zcs[p][:], av[:])
                    else:
                        nc.vector.tensor_add(zcs[p][:], zcs[p][:], av[:])
            for p, r0 in prs:
                zc = zcs[p]
                zlo = s_z.tile([64, 512], F32, name="zlo", tag=f"zlo{r0}")
                nc.sync.dma_start(zlo[:], zc[64:128, :])
                rzlo = s_z.tile([64, 512], F32, name="rzlo", tag=f"rzlo{r0}")
                nc.vector.reciprocal_approx_fast(rzlo[:], zlo[:])
                if r0 == 0:
                    nc.vector.tensor_mul(ctxT[0:64, g, i0:i0 + 512],
                                         zc[0:64, :], rzlo[:])
                else:
                    cstg = s_z.tile([64, 512], F32R, name="cstg", tag="cstg")
                    nc.vector.tensor_mul(cstg[:], zc[0:64, :], rzlo[:])
                    nc.sync.dma_start(ctxT[64:128, g, i0:i0 + 512], cstg[:])

        def phase_b_it(g, it):
            prs = prs_of(g)
            i0 = it * 128
            Ps, zqps = {}, {}
            for p, r0 in prs:
                Ps[p] = s_p.tile([128, N], F32, name="P", tag=f"P{r0}")
                zqps[p] = s_z.tile([128, NJG], F32, name="zqp", tag=f"zqp{r0}")
            for jg in range(NJG):
                j0 = jg * 1024
                w = min(1024, N - j0)
                qms = {}
                for p, r0 in prs:
                    qms[p] = p_s.tile([128, 1024], F32, name="qm", tag="slab")
                for j in range(w // 512):
                    for p, r0 in prs:
                        nc.tensor.matmul(
                            qms[p][:, j * 512:(j + 1) * 512],
                            qT[r0:r0 + 64, g, i0:i0 + 128],
                            kT[r0:r0 + 64, g, j0 + j * 512:j0 + (j + 1) * 512],
                            start=True, stop=True)
                for p, r0 in prs:
                    nc.scalar.activation(Ps[p][:, j0:j0 + w],
                                         qms[p][:, 0:w], AF.Exp,
                                         accum_out=zqps[p][:, jg:jg + 1])
            for p, r0 in prs:
                zq = s_z.tile([128, 1], F32, name="zq", tag=f"zq{r0}")
                rzq = s_z.tile([128, 1], F32, name="rzq", tag=f"rzq{r0}")
                if NJG > 1:
                    nc.vector.tensor_reduce(zq[:], zqps[p][:],
                                            mybir.AxisListType.X,
                                            mybir.AluOpType.add)
                else:
                    zq = zqps[p]
                nc.vector.reciprocal(rzq[:], zq[:])
                nc.vector.tensor_scalar_mul(Ps[p][:], Ps[p][:], rzq[:])
                nc.sync.dma_start(attn_d[p, i0:i0 + 128, :], Ps[p][:])

        # schedule: per group, interleave A-ibs with B-its (B is independent
        # of A thanks to accum_out Z); wout rides with group 1
        for step in range(NIB):
            phase_a_ib(0, step)
            for it in range(step * (MT // NIB), (step + 1) * (MT // NIB)):
                phase_b_it(0, it)

        # ---- stage 5 interleaved into B(1): out partial = ctxT.T @ wo ----
        nc.gpsimd.dma_start(wo_sb[:], wo_d.rearrange("(g p) n -> p g n", p=128))
        s_o = ctx.enter_context(tc.tile_pool(name="s_o", bufs=3))

        def wout_it(it):
            i0 = it * 128
            for nt in range((D + 511) // 512):
                nf = min(512, D - nt * 512)
                po = p_av.tile([128, 512], F32, name="po", tag="av0")
                for ct in range(NG):
                    nc.tensor.matmul(po[:, 0:nf], ctxT[:, ct, i0:i0 + 128],
                                     wo_sb[:, ct, nt * 512:nt * 512 + nf],
                                     start=(ct == 0), stop=(ct == NG - 1))
                osb = s_o.tile([128, 512], F32, name="osb", tag="osb")
                nc.vector.tensor_copy(osb[:, 0:nf], po[:, 0:nf])
                nc.sync.dma_start(outp_d[i0:i0 + 128, nt * 512:nt * 512 + nf],
                                  osb[:, 0:nf])

        for step in range(NIB):
            phase_a_ib(1, step)
            for it in range(step * (MT // NIB), (step + 1) * (MT // NIB)):
                phase_b_it(1, it)
                wout_it(it)

        if dbg_dump:
            nc.gpsimd.dma_start(qT_d[:], qT[:])
            nc.gpsimd.dma_start(kT_d[:], kT[:])
            nc.gpsimd.dma_start(v_d[:], v_sb[:])
            nc.gpsimd.dma_start(ctx_d[:], ctxT[:])

        ctx.close()

    nc.compile()
    return nc


def host_prep(cfg: Cfg, x, freqs_cis, W_qkv, W_out):
    """Build per-core input maps."""
    N, D, NH, HD = cfg.N, cfg.D, cfg.NH, cfg.HD
    x = np.ascontiguousarray(np.asarray(x, dtype=np.float32))
    freqs_cis = np.asarray(freqs_cis, dtype=np.float32)
    W_qkv = np.asarray(W_qkv, dtype=np.float32)
    W_out = np.asarray(W_out, dtype=np.float32)
    scale = float(D) ** -0.5
    D2 = HD // 2

    # rope multiplier tiles [128, N]: 4 blocks of 32 rows
    cosT = np.ascontiguousarray(freqs_cis[:, :, 0].T)   # [D2=32, N]
    sinT = np.ascontiguousarray(freqs_cis[:, :, 1].T)
    assert D2 == 32
    rcos = np.ascontiguousarray(np.tile(cosT, (4, 1)))
    # pre-swapped sign layout: after multiplying, a +-32-partition block swap
    # (done via SBUF->SBUF DMA) lands each product on its target rows
    rsin = np.ascontiguousarray(np.concatenate([sinT, -sinT, sinT, -sinT], axis=0))

    perm = np.concatenate([np.arange(0, HD, 2), np.arange(1, HD, 2)])  # evens|odds

    total_heads = W_qkv.shape[1] // (3 * HD) if False else (
        W_qkv.shape[1] // 3 // HD)
    n_hg = total_heads // NH  # head groups per batch
    in_maps = []
    for c in range(cfg.n_cores):
        b, hg = divmod(c, n_hg)
        heads = range(hg * NH, (hg + 1) * NH)
        qcols, kcols, vcols, orows = [], [], [], []
        for h in heads:
            base = h * HD
            qcols.append(base + perm)
            kcols.append(D + base + perm)
            vcols.append(2 * D + base + np.arange(HD))
            orows.append(base + np.arange(HD))
        qcols = np.concatenate(qcols)
        kcols = np.concatenate(kcols)
        vcols = np.concatenate(vcols)
        orows = np.concatenate(orows)
        in_maps.append({
            "xb": x[b],
            "wq": np.ascontiguousarray(W_qkv[:, qcols] * scale),
            "wk": np.ascontiguousarray(W_qkv[:, kcols]),
            "wv": np.ascontiguousarray(W_qkv[:, vcols]),
            "wo": np.ascontiguousarray(W_out[orows, :]),
            "rcos": rcos,
            "rsin": rsin,
            "ones": np.ones((128, 256), dtype=np.float32),
        })
    return in_maps


_NC_CACHE = {}


def kernel(x, freqs_cis, W_qkv, W_out, b_out):
    from concourse.bass_utils import run_bass_kernel_spmd

    cfg = CFG
    in_maps = host_prep(cfg, x, freqs_cis, W_qkv, W_out)

    if "nc" not in _NC_CACHE:
        _NC_CACHE["nc"] = build_bass(cfg)
    nc = _NC_CACHE["nc"]

    kwargs = {}
    if os.environ.get("BASS_KERNEL_TRACE"):
        kwargs = dict(trace=True, trace_cores=[0])
    res = run_bass_kernel_spmd(nc, in_maps, core_ids=list(range(cfg.n_cores)),
                               **kwargs)
    _NC_CACHE["last_results"] = res

    N, D, NH = cfg.N, cfg.D, cfg.NH
    n_hg = 16 // NH
    b_out = np.asarray(b_out, dtype=np.float32)
    attn = np.empty((2, 16, N, N), dtype=np.float32)
    out = np.zeros((2, N, D), dtype=np.float32)
    for c in range(cfg.n_cores):
        b, hg = divmod(c, n_hg)
        r = res.results[c]
        attn[b, hg * NH:(hg + 1) * NH] = r["attn4"]
        out[b] += r["outp"]
    out += b_out
    return (out, attn)


# revision 47
# speedup vs baseline: 1.0164x; 1.0094x over previous
"""Trainium2 Bass kernel for nn_Attention_61040075210876.

Full multi-head attention block: qkv = x @ W_qkv, RoPE on q/k,
attn = softmax(q k^T / sqrt(D)), ctx = attn @ v, out = ctx @ W_out + b_out.
Returns (out, attn) like the reference.

Sharding over 8 NeuronCores: core c handles batch b = c//4 and the 4 heads
h in [4*(c%4), 4*(c%4)+4). Per core:
  - qkv projection only for its heads (W_qkv column shard; q-columns
    pre-scaled by D**-0.5; q/k columns permuted per head to [evens|odds]
    so RoPE pair-partners sit 32 partitions apart in feature-major layout)
  - scores computed TWICE on the tensor engine (query-major for the attn
    output, with row-sum Z from the exp activation's accum_out; key-major
    for the attn @ v contraction, with 64 ones-columns appended to v so the
    same matmul replicates Z across partitions 64:128)
  - the attn@v accumulation runs as two closed 8-matmul PSUM groups (HW
    mis-accumulates when an accumulation group interleaves with other
    matmul groups, though CoreSim accepts it); the per-key reciprocal runs
    at base partition 0 after a DMA lane shift (custom-DVE ops at base 64
    also misbehaved on HW)
  - out partial = ctx @ W_out(row shard); host sums partials + bias.

All heavy matmuls run in float32r (fp32 data rounded to fp32r by the
producing instruction; ~1.5e-4 rel err, 4x faster than true fp32).
"""
import sys
import os
from contextlib import ExitStack
from dataclasses import dataclass

sys.path.insert(0, "/opt/trn_rl_repo")

import numpy as np

import concourse.bass as bass  # noqa: E402
import concourse.tile as tile  # noqa: E402
from concourse import bacc, mybir  # noqa: E402
from concourse.masks import make_identity  # noqa: E402

F32 = mybir.dt.float32
F32R = mybir.dt.float32r
BF16 = mybir.dt.bfloat16
AF = mybir.ActivationFunctionType


@dataclass(frozen=True)
class Cfg:
    N: int = 2048      # sequence length
    D: int = 1024      # model dim
    NH: int = 4        # heads per core
    HD: int = 64       # head dim
    n_cores: int = 8

    @property
    def DH(self):      # features per core for q/k/v
        return self.NH * self.HD

    @property
    def KT(self):      # 128-row tiles of D
        return self.D // 128

    @property
    def MT(self):      # 128-row tiles of N
        return self.N // 128

    @property
    def NG(self):      # 128-row feature groups (2 heads each)
        return self.DH // 128


CFG = Cfg()


def build_bass(cfg: Cfg, debug: bool = False, dbg_dump: bool = False, reps: int = 1):
    nc = bacc.Bacc("TRN2", target_bir_lowering=False, debug=debug)
    N, D, NH, HD = cfg.N, cfg.D, cfg.NH, cfg.HD
    DH, KT, MT, NG = cfg.DH, cfg.KT, cfg.MT, cfg.NG
    assert N % 256 == 0 and D % 128 == 0 and HD == 64 and NH % 2 == 0

    xb_d = nc.dram_tensor("xb", [N, D], F32, kind="ExternalInput").ap()
    wq_d = nc.dram_tensor("wq", [D, DH], F32, kind="ExternalInput").ap()
    wk_d = nc.dram_tensor("wk", [D, DH], F32, kind="ExternalInput").ap()
    wv_d = nc.dram_tensor("wv", [D, DH], F32, kind="ExternalInput").ap()
    wo_d = nc.dram_tensor("wo", [DH, D], F32, kind="ExternalInput").ap()
    rcos_d = nc.dram_tensor("rcos", [128, N], F32, kind="ExternalInput").ap()
    rsin_d = nc.dram_tensor("rsin", [128, N], F32, kind="ExternalInput").ap()
    ones_d = nc.dram_tensor("ones", [128, 256], F32, kind="ExternalInput").ap()
    attn_d = nc.dram_tensor("attn4", [NH, N, N], F32, kind="ExternalOutput").ap()
    outp_d = nc.dram_tensor("outp", [N, D], F32, kind="ExternalOutput").ap()
    if dbg_dump:
        qT_d = nc.dram_tensor("qT_dbg", [128, cfg.NG, N], F32, kind="ExternalOutput").ap()
        kT_d = nc.dram_tensor("kT_dbg", [128, cfg.NG, N], F32, kind="ExternalOutput").ap()
        v_d = nc.dram_tensor("v_dbg", [128, cfg.MT, NH, 2 * HD], F32, kind="ExternalOutput").ap()
        ctx_d = nc.dram_tensor("ctx_dbg", [128, cfg.NG, N], F32, kind="ExternalOutput").ap()

    with tile.TileContext(nc) as tc, ExitStack() as octx:
      for _rep in range(reps):
        ctx = octx.enter_context(ExitStack())
        # ---- persistent tiles ----
        pers = ctx.enter_context(tc.tile_pool(name="pers", bufs=1))
        qT = pers.tile([128, NG, N], F32R, name="qT")     # rope'd, feature-major
        kT = pers.tile([128, NG, N], F32R, name="kT")
        v_sb = pers.tile([128, MT, NH, 2 * HD], F32R, name="v_sb")
        ctxT = pers.tile([128, NG, N], F32R, name="ctxT")
        wo_sb = pers.tile([128, NG, D], F32R, name="wo_sb")

        for _mt in range(MT):
            nc.gpsimd.dma_start(
                v_sb[:, _mt, :, HD:2 * HD],
                ones_d.rearrange("p (h d) -> p h d", d=64))

        # ---- stage 1+2+3: x transpose (halves), v, per-group q/k + rope ----
        with ExitStack() as sctx:
            s_in = sctx.enter_context(tc.tile_pool(name="s_in", bufs=3))
            s_w = sctx.enter_context(tc.tile_pool(name="s_w", bufs=1))
            s_xt = sctx.enter_context(tc.tile_pool(name="s_xt", bufs=1))
            s_tmp = sctx.enter_context(tc.tile_pool(name="s_tmp", bufs=2))
            p_tr = sctx.enter_context(tc.tile_pool(name="p_tr", bufs=4, space="PSUM"))
            p_qk = sctx.enter_context(tc.tile_pool(name="p_qk", bufs=2, space="PSUM"))
            p_v = sctx.enter_context(tc.tile_pool(name="p_v", bufs=2, space="PSUM"))

            ident = s_w.tile([128, 128], F32, name="ident")
            make_identity(nc, ident)

            wq_sb = s_w.tile([128, KT, DH], F32R, name="wq_sb")
            wk_sb = s_w.tile([128, KT, DH], F32R, name="wk_sb")
            wv_sb = s_w.tile([128, KT, DH], F32R, name="wv_sb")
            rcos = s_w.tile([128, N], F32R, name="rcos")
            rsin = s_w.tile([128, N], F32R, name="rsin")
            for w_d, w_sb in ((wv_d, wv_sb), (wq_d, wq_sb), (wk_d, wk_sb)):
                wstg = s_w.tile([128, KT, DH], F32, name="wstg", tag="wstg")
                nc.sync.dma_start(wstg[:], w_d.rearrange("(t p) n -> p t n", p=128))
                nc.vector.tensor_copy(w_sb[:], wstg[:])
            nc.gpsimd.dma_start(rcos[:], rcos_d[:])
            nc.gpsimd.dma_start(rsin[:], rsin_d[:])

            xb_r = xb_d.rearrange("(mt p) d -> mt p d", p=128)
            NH2 = N // 2
            MH = MT // 2

            def rope_group(dstT, g):
                # in place: t = t*cos + swap32(t*sin')  (sin' pre-swapped signs)
                for h0 in range(0, N, N // 2):
                    cols = slice(h0, h0 + N // 2)
                    dst = dstT[:, g, cols]
                    tmp = s_tmp.tile([128, N // 2], F32R, name="tmp", tag="tmp")
                    tmp2 = s_tmp.tile([128, N // 2], F32R, name="tmp2",
                                      tag="tmp2")
                    nc.vector.tensor_mul(tmp[:], dst, rsin[:, cols])
                    for s in range(4):
                        lo, hi = s * 32, (s + 1) * 32
                        plo, phi = (s ^ 1) * 32, ((s ^ 1) + 1) * 32
                        nc.sync.dma_start(tmp2[lo:hi, :], tmp[plo:phi, :])
                    nc.vector.tensor_mul(dst, dst, rcos[:, cols])
                    nc.vector.tensor_add(dst, dst, tmp2[:])

            for half in range(2):
                xTh = s_xt.tile([128, KT, NH2], F32R, name="xTh", tag="xTh")
                for mt in range(MH):
                    gmt = half * MH + mt
                    xrow = s_in.tile([128, D], F32, name="xrow", tag="xrow")
                    nc.sync.dma_start(xrow[:], xb_r[gmt])
                    for dt0 in range(0, KT, 4):
                        nd = min(4, KT - dt0)
                        ptr = p_tr.tile([128, 512], F32, name="ptr")
                        for dj in range(nd):
                            nc.tensor.transpose(
                                ptr[:, dj * 128:(dj + 1) * 128],
                                xrow[:, (dt0 + dj) * 128:(dt0 + dj + 1) * 128],
                                ident[:])
                        nc.vector.tensor_copy(
                            xTh[:, dt0:dt0 + nd, mt * 128:(mt + 1) * 128],
                            ptr[:, 0:nd * 128].rearrange("p (d c) -> p d c",
                                                         c=128))

                IC = min(512, NH2)

                def qk_group(g):
                    for w_sb, dstT in ((wk_sb, kT), (wq_sb, qT)):
                        for ic in range(NH2 // IC):
                            pq = p_qk.tile([128, 512], F32, name="pq")
                            for t in range(KT):
                                nc.tensor.matmul(
                                    pq[:, 0:IC],
                                    w_sb[:, t, g * 128:(g + 1) * 128],
                                    xTh[:, t, ic * IC:(ic + 1) * IC],
                                    start=(t == 0), stop=(t == KT - 1))
                            nc.vector.tensor_copy(
                                dstT[:, g, half * NH2 + ic * IC:
                                     half * NH2 + (ic + 1) * IC], pq[:, 0:IC])
                    if half == 1:
                        rope_group(kT, g)
                        rope_group(qT, g)

                # g0 first so its rope (gating stage 4) lands earliest;
                # v before g1 (AV needs it early in phase A)
                qk_group(0)
                for mt in range(MH):
                    gmt = half * MH + mt
                    pv = p_v.tile([128, DH], F32, name="pv")
                    for t in range(KT):
                        nc.tensor.matmul(
                            pv[:], xTh[:, t, mt * 128:(mt + 1) * 128],
                            wv_sb[:, t, :], start=(t == 0), stop=(t == KT - 1))
                    nc.vector.tensor_copy(
                        v_sb[:, gmt, :, 0:HD],
                        pv[:].rearrange("p (h d) -> p h d", d=HD))
                for g in range(1, NG):
                    qk_group(g)

        # ---- stage 4: attention, pairs of a group interleaved for PE
        # row/col-group packing (even pair on partitions 0:64, odd on 64:128)
        s_et = ctx.enter_context(tc.tile_pool(name="s_et", bufs=1))
        s_p = ctx.enter_context(tc.tile_pool(name="s_p", bufs=2))
        s_z = ctx.enter_context(tc.tile_pool(name="s_z", bufs=2))
        p_s = ctx.enter_context(tc.tile_pool(name="p_s", bufs=3, space="PSUM"))
        p_av = ctx.enter_context(tc.tile_pool(name="p_av", bufs=1, space="PSUM"))

        NJT = MT                  # key-major j-tiles (128 wide)
        NIB = N // 512            # key-major i-blocks
        NJG = (N + 1023) // 1024  # 1024-wide psum slabs per row


        def prs_of(g):
            return [(2 * g, 0), (2 * g + 1, 64)]

        def phase_a_ib(g, ib):
            prs = prs_of(g)
            i0 = ib * 512
            NSEG = 2
            seg_len = NJT // NSEG
            # expT for one segment of j-tiles, per pair
            ets = {p: [] for p, _ in prs}
            avsegs = {p: [] for p, _ in prs}
            zcs = {}
            for p, r0 in prs:
                zcs[p] = s_z.tile([128, 512], F32, name="zc", tag=f"zc{r0}")
            for seg in range(NSEG):
                for p, r0 in prs:
                    ets[p].append(s_et.tile([128, seg_len * 512], F32R,
                                            name="et", tag=f"et{r0}"))
                for jt0 in range(seg * seg_len, (seg + 1) * seg_len, 2):
                    nj = min(2, (seg + 1) * seg_len - jt0)
                    slabs = {}
                    for p, r0 in prs:
                        slabs[p] = p_s.tile([128, 1024], F32, name="km",
                                            tag="slab")
                    for j in range(nj):
                        jt = jt0 + j
                        for p, r0 in prs:
                            nc.tensor.matmul(
                                slabs[p][:, j * 512:(j + 1) * 512],
                                kT[r0:r0 + 64, g, jt * 128:(jt + 1) * 128],
                                qT[r0:r0 + 64, g, i0:i0 + 512],
                                start=True, stop=True)
                    for p, r0 in prs:
                        c0 = (jt0 - seg * seg_len) * 512
                        nc.scalar.activation(ets[p][-1][:, c0:c0 + nj * 512],
                                             slabs[p][:, 0:nj * 512], AF.Exp)
                # closed accumulation group: seg_len consecutive matmuls,
                # folded into the SBUF accumulator right away so the av psum
                # slot recycles between segments (frees 2 banks for slabs)
                for p, r0 in prs:
                    av = p_av.tile([128, 512], F32, name="av", tag=f"av{r0}")
                    avsegs[p].append(av)
                    for jj in range(seg_len):
                        jt = seg * seg_len + jj
                        nc.tensor.matmul(av[:], v_sb[:, jt, p, :],
                                         ets[p][-1][:, jj * 512:(jj + 1) * 512],
                                         start=(jj == 0),
                                         stop=(jj == seg_len - 1))
                    if seg == 0:
                        nc.vector.tensor_copy(zcs[p][:], av[:])
                    else:
                        nc.vector.tensor_add(zcs[p][:], zcs[p][:], av[:])
            for p, r0 in prs:
                zc = zcs[p]
                zlo = s_z.tile([64, 512], F32, name="zlo", tag=f"zlo{r0}")
                nc.sync.dma_start(zlo[:], zc[64:128, :])
                rzlo = s_z.tile([64, 512], F32, name="rzlo", tag=f"rzlo{r0}")
                nc.vector.reciprocal_approx_fast(rzlo[:], zlo[:])
                if r0 == 0:
                    nc.vector.tensor_mul(ctxT[0:64, g, i0:i0 + 512],
                                         zc[0:64, :], rzlo[:])
                else:
                    cstg = s_z.tile([64, 512], F32R, name="cstg", tag="cstg")
                    nc.vector.tensor_mul(cstg[:], zc[0:64, :], rzlo[:])
                    nc.sync.dma_start(ctxT[64:128, g, i0:i0 + 512], cstg[:])

        def phase_b_it(g, it):
            prs = prs_of(g)
            i0 = it * 128
            Ps, zqps = {}, {}
            for p, r0 in prs:
                Ps[p] = s_p.tile([128, N], F32, name="P", tag=f"P{r0}")
                zqps[p] = s_z.tile([128, NJG], F32, name="zqp", tag=f"zqp{r0}")
            for jg in range(NJG):
                j0 = jg * 1024
                w = min(1024, N - j0)
                qms = {}
                for p, r0 in prs:
                    qms[p] = p_s.tile([128, 1024], F32, name="qm", tag="slab")
                for j in range(w // 512):
                    for p, r0 in prs:
                        nc.tensor.matmul(
                            qms[p][:, j * 512:(j + 1) * 512],
                            qT[r0:r0 + 64, g, i0:i0 + 128],
                            kT[r0:r0 + 64, g, j0 + j * 512:j0 + (j + 1) * 512],
                            start=True, stop=True)
                for p, r0 in prs:
                    nc.scalar.activation(Ps[p][:, j0:j0 + w],
                                         qms[p][:, 0:w], AF.Exp,
                                         accum_out=zqps[p][:, jg:jg + 1])
            for p, r0 in prs:
                zq = s_z.tile([128, 1], F32, name="zq", tag=f"zq{r0}")
                rzq = s_z.tile([128, 1], F32, name="rzq", tag=f"rzq{r0}")
                if NJG > 1:
                    nc.vector.tensor_reduce(zq[:], zqps[p][:],
                                            mybir.AxisListType.X,
                                            mybir.AluOpType.add)
                else:
                    zq = zqps[p]
                nc.vector.reciprocal(rzq[:], zq[:])
                nc.vector.tensor_scalar_mul(Ps[p][:], Ps[p][:], rzq[:])
                nc.sync.dma_start(attn_d[p, i0:i0 + 128, :], Ps[p][:])

        # schedule: per group, interleave A-ibs with B-its (B is independent
        # of A thanks to accum_out Z); wout rides with group 1
        for step in range(NIB):
            for it in range(step * (MT // NIB), (step + 1) * (MT // NIB)):
                phase_b_it(0, it)
            phase_a_ib(0, step)

        # ---- stage 5 interleaved into B(1): out partial = ctxT.T @ wo ----
        nc.gpsimd.dma_start(wo_sb[:], wo_d.rearrange("(g p) n -> p g n", p=128))
        s_o = ctx.enter_context(tc.tile_pool(name="s_o", bufs=3))

        def wout_it(it):
            i0 = it * 128
            for nt in range((D + 511) // 512):
                nf = min(512, D - nt * 512)
                po = p_av.tile([128, 512], F32, name="po", tag="av0")
                for ct in range(NG):
                    nc.tensor.matmul(po[:, 0:nf], ctxT[:, ct, i0:i0 + 128],
                                     wo_sb[:, ct, nt * 512:nt * 512 + nf],
                                     start=(ct == 0), stop=(ct == NG - 1))
                osb = s_o.tile([128, 512], F32, name="osb", tag="osb")
                nc.vector.tensor_copy(osb[:, 0:nf], po[:, 0:nf])
                nc.sync.dma_start(outp_d[i0:i0 + 128, nt * 512:nt * 512 + nf],
                                  osb[:, 0:nf])

        for step in range(NIB):
            phase_a_ib(1, step)
            for it in range(step * (MT // NIB), (step + 1) * (MT // NIB)):
                phase_b_it(1, it)
                wout_it(it)

        if dbg_dump:
            nc.gpsimd.dma_start(qT_d[:], qT[:])
            nc.gpsimd.dma_start(kT_d[:], kT[:])
            nc.gpsimd.dma_start(v_d[:], v_sb[:])
            nc.gpsimd.dma_start(ctx_d[:], ctxT[:])

        ctx.close()

    nc.compile()
    return nc


def host_prep(cfg: Cfg, x, freqs_cis, W_qkv, W_out):
    """Build per-core input maps."""
    N, D, NH, HD = cfg.N, cfg.D, cfg.NH, cfg.HD
    x = np.ascontiguousarray(np.asarray(x, dtype=np.float32))
    freqs_cis = np.asarray(freqs_cis, dtype=np.float32)
    W_qkv = np.asarray(W_qkv, dtype=np.float32)
    W_out = np.asarray(W_out, dtype=np.float32)
    scale = float(D) ** -0.5
    D2 = HD // 2

    # rope multiplier tiles [128, N]: 4 blocks of 32 rows
    cosT = np.ascontiguousarray(freqs_cis[:, :, 0].T)   # [D2=32, N]
    sinT = np.ascontiguousarray(freqs_cis[:, :, 1].T)
    assert D2 == 32
    rcos = np.ascontiguousarray(np.tile(cosT, (4, 1)))
    # pre-swapped sign layout: after multiplying, a +-32-partition block swap
    # (done via SBUF->SBUF DMA) lands each product on its target rows
    rsin = np.ascontiguousarray(np.concatenate([sinT, -sinT, sinT, -sinT], axis=0))

    perm = np.concatenate([np.arange(0, HD, 2), np.arange(1, HD, 2)])  # evens|odds

    total_heads = W_qkv.shape[1] // (3 * HD) if False else (
        W_qkv.shape[1] // 3 // HD)
    n_hg = total_heads // NH  # head groups per batch
    in_maps = []
    for c in range(cfg.n_cores):
        b, hg = divmod(c, n_hg)
        heads = range(hg * NH, (hg + 1) * NH)
        qcols, kcols, vcols, orows = [], [], [], []
        for h in heads:
            base = h * HD
            qcols.append(base + perm)
            kcols.append(D + base + perm)
            vcols.append(2 * D + base + np.arange(HD))
            orows.append(base + np.arange(HD))
        qcols = np.concatenate(qcols)
        kcols = np.concatenate(kcols)
        vcols = np.concatenate(vcols)
        orows = np.concatenate(orows)
        in_maps.append({
            "xb": x[b],
            "wq": np.ascontiguousarray(W_qkv[:, qcols] * scale),
            "wk": np.ascontiguousarray(W_qkv[:, kcols]),
            "wv": np.ascontiguousarray(W_qkv[:, vcols]),
            "wo": np.ascontiguousarray(W_out[orows, :]),
            "rcos": rcos,
            "rsin": rsin,
            "ones": np.ones((128, 256), dtype=np.float32),
        })
    return in_maps


_NC_CACHE = {}


def kernel(x, freqs_cis, W_qkv, W_out, b_out):
    from concourse.bass_utils import run_bass_kernel_spmd

    cfg = CFG
    in_maps = host_prep(cfg, x, freqs_cis, W_qkv, W_out)

    if "nc" not in _NC_CACHE:
        _NC_CACHE["nc"] = build_bass(cfg)
    nc = _NC_CACHE["nc"]

    kwargs = {}
    if os.environ.get("BASS_KERNEL_TRACE"):
        kwargs = dict(trace=True, trace_cores=[0])
    res = run_bass_kernel_spmd(nc, in_maps, core_ids=list(range(cfg.n_cores)),
                               **kwargs)
    _NC_CACHE["last_results"] = res

    N, D, NH = cfg.N, cfg.D, cfg.NH
    n_hg = 16 // NH
    b_out = np.asarray(b_out, dtype=np.float32)
    attn = np.empty((2, 16, N, N), dtype=np.float32)
    out = np.zeros((2, N, D), dtype=np.float32)
    for c in range(cfg.n_cores):
        b, hg = divmod(c, n_hg)
        r = res.results[c]
        attn[b, hg * NH:(hg + 1) * NH] = r["attn4"]
        out[b] += r["outp"]
    out += b_out
    return (out, attn)
